# revision 11
# baseline (speedup 1.0000x reference)
"""Trainium2 Bass kernel for blended-expert 3-layer MLP (moe_routing).

Math (per sample b):
  h1 = elu(sum_e blend[e,b] * (W1[e] @ x[b]  + b1[e]))
  h2 = elu(sum_e blend[e,b] * (W2[e] @ h1[b] + b2[e]))
  y  = softmax(sum_e blend[e,b] * (W3[e] @ h2[b] + b3[e]))

Strategy (per core, data-parallel over batch: B=8192 -> Bc=1024 per core):
  - fp16 everywhere on the matmul path (10-bit mantissa ~ fp32r's 11):
    weights live fp16 and fully SBUF-RESIDENT (13.3MB), loaded once in the
    prologue, so the steady-state loop does no weight DMA at all.
  - Activations live TRANSPOSED in SBUF as fp16: hT[d, b] (d on partitions).
    Host pre-transposes x; host un-transposes the [363, Bc] output.
  - Blended linear as one PSUM accumulation: for each expert e the moving
    operand is rhs_e = hT * blend[e, :] (DVE tensor_tensor in fp16 2x mode
    with a host-replicated broadcast tile), the stationary is a chunk of
    W_e^T (fp16, 1 cycle/row on PE - same as fp32r).
  - L1 bias rides the padded K dim: x is padded 480->512 with row 480 = 1,
    and the W1 pack puts b1[e] in row 480, so rhs row 480 = blend[e,:] and
    the bias accumulates inside the main matmuls for free.
  - L2/L3 blended bias enters the PSUM group via a K=8 fp16 matmul
    (stationary = bias matrix [8, out], moving = blend [8, b]).
  - ELU drain: ACT exp + DVE tensor_scalar/scalar_tensor_tensor:
      elu(v) = max(v, 0) + min(exp(v) - 1, 0)     -> written as fp16
  - Softmax stays fp32 (exp can exceed fp16 range): exp via ACT, partition
    sums via a ones-stationary f32r matmul. The normalization (reciprocal,
    GPSIMD partition-broadcast, scale, DMA out) is SKEWED one iteration:
    iteration i banks its exp tiles and sums in SBUF, iteration i+1 (or an
    epilogue flush) normalizes them while its own L1 matmuls run, so the
    tail never blocks the PE or the DVE rhs pipeline.
  - Software pipelining: each layer's last expert runs ot-outer so every
    accumulator finishes a full K-round early; its ELU drain and the next
    layer's e0 rhs build pipeline behind the remaining matmul blocks. With
    this the sim shows the PE 100% busy in steady state (~156us/rep vs the
    ~170us/rep of the fp32r streaming baseline).
  - PSUM: 8 banks exactly. The last-ot accumulator of each layer is seeded
    and first-touched only after a full e0 K-round of the other accumulators,
    so the two banks that the previous softmax sums occupy have time to free
    without stalling the PE at layer/rep boundaries.
"""

import numpy as np

import concourse.bass as bass
import concourse.mybir as mybir
import concourse.tile as tile
from concourse import bacc
from concourse.bass_utils import run_bass_kernel_spmd

F32 = mybir.dt.float32
F32R = mybir.dt.float32r
F16 = mybir.dt.float16
AF = mybir.ActivationFunctionType
OP = mybir.AluOpType

N_CORES = 8
E = 8
B = 8192
BC = B // N_CORES          # 1024 per core
BT = 2                     # batch halves per core (PSUM free dim = 512)
BW = BC // BT              # 512
D0, D1, D2, D3 = 480, 512, 512, 363
D0P = 512                  # input dim padded to 4 K-chunks of 128
KC = 4                     # K chunks of 128 per expert (all layers)
# layer table: (out_dim, n_otiles, bias column offset into bias tile or None)
LAYERS = [(D1, 4, None), (D2, 4, 0), (D3, 3, D2)]
BIAS_W = D2 + D3           # b1 is folded into the W1 pack


def _build_program(reps=1, unroll=False):
    nc = bacc.Bacc("TRN2", target_bir_lowering=False, debug=False,
                   num_devices=N_CORES)

    xt_d = nc.dram_tensor("xt", [128, KC * BC], F16, kind="ExternalInput").ap()
    bc_d = nc.dram_tensor("bcast", [128, E * BC], F16, kind="ExternalInput").ap()
    bl_d = nc.dram_tensor("blend", [E, BC], F16, kind="ExternalInput").ap()
    bias_d = nc.dram_tensor("bias", [E, BIAS_W], F16, kind="ExternalInput").ap()
    ones_d = nc.dram_tensor("ones", [128, 1], F32R, kind="ExternalInput").ap()
    w_d = [
        nc.dram_tensor("w1", [128, E * KC * D1], F16, kind="ExternalInput").ap(),
        nc.dram_tensor("w2", [128, E * KC * D2], F16, kind="ExternalInput").ap(),
        nc.dram_tensor("w3", [128, E * KC * D3], F16, kind="ExternalInput").ap(),
    ]
    y_d = nc.dram_tensor("y", [D3, BC], F32, kind="ExternalOutput").ap()

    with tile.TileContext(nc) as tc:
        with (
            tc.tile_pool(name="const", bufs=1) as cpool,
            tc.tile_pool(name="acts", bufs=1) as apool,
            tc.tile_pool(name="rhs", bufs=10) as rpool,
            tc.tile_pool(name="drain", bufs=4) as dpool,
            tc.tile_pool(name="psum", bufs=8, space="PSUM") as ppool,
        ):
            xt = cpool.tile([128, KC, BC], F16)
            bcast = cpool.tile([128, E, BC], F16)
            blend = cpool.tile([E, BC], F16)
            bias = cpool.tile([E, BIAS_W], F16)
            ones = cpool.tile([128, 1], F32R)
            ws = [
                cpool.tile([128, E * KC * D1], F16, name="w1s"),
                cpool.tile([128, E * KC * D2], F16, name="w2s"),
                cpool.tile([128, E * KC * D3], F16, name="w3s"),
            ]
            # prologue: everything is resident; spread the ~13MB of loads
            # over both the SP and Activation HW-DGE queues
            nc.sync.dma_start(out=blend[:], in_=bl_d[:])
            nc.sync.dma_start(out=bias[:], in_=bias_d[:])
            nc.sync.dma_start(out=ones[:], in_=ones_d[:])
            nc.sync.dma_start(out=xt[:], in_=xt_d[:])
            nc.sync.dma_start(out=bcast[:], in_=bc_d[:])
            nc.sync.dma_start(out=ws[0][:], in_=w_d[0][:])
            nc.scalar.dma_start(out=ws[1][:], in_=w_d[1][:])
            nc.scalar.dma_start(out=ws[2][:], in_=w_d[2][:])

            h1 = apool.tile([128, KC, BC], F16)
            h2 = apool.tile([128, KC, BC], F16)
            srcs = [xt, h1, h2]
            # persistent softmax state: iteration i's exp tiles / sums are
            # normalized and written out at the START of iteration i+1 (the
            # DVE is idle there), with one epilogue flush after the loop
            exs_p = [[apool.tile([128, BW], F32, name=f"exp_b{bt}_o{ot}")
                      for ot in range(LAYERS[2][1])] for bt in range(BT)]
            ssb_p = [apool.tile([1, BW], F32, name=f"ssb_b{bt}")
                     for bt in range(BT)]
            for bt in range(BT):
                nc.gpsimd.memset(ssb_p[bt][:], 1.0)
                for ot in range(LAYERS[2][1]):
                    nc.gpsimd.memset(exs_p[bt][ot][:], 0.0)

            def body():
                _network(nc, tc, srcs, bcast, blend, bias, ones,
                         ws, y_d, rpool, dpool, ppool, exs_p, ssb_p)

            if reps == 1:
                body()
            elif unroll:
                for _ in range(reps):
                    body()
            else:
                with tc.For_i(0, reps, 1):
                    body()
            _softmax_tail(nc, dpool, exs_p, ssb_p, y_d)
    nc.compile()
    return nc


def _network(nc, tc, srcs, bcast, blend, bias, ones, ws, y_d,
             rpool, dpool, ppool, exs_p, ssb_p):
    tail_done = False
    prebuilt = None          # e0 rhs tiles built during the previous layer
    for li, (dout, n_ot, boff) in enumerate(LAYERS):
        src = srcs[li]
        w = ws[li]
        ol = n_ot - 1          # deferred (last) output tile

        def wsl(e, kc, ot, otw):
            base = (e * KC + kc) * dout + ot * 128
            return w[:, base:base + otw]

        def build_rhs(e, kc, name):
            rhs = rpool.tile([128, BC], F16, tag="rhs", name=name)
            nc.vector.tensor_tensor(rhs[:], src[:, kc, :], bcast[:, e, :],
                                    OP.mult)
            return rhs

        # psum accumulators: one bank per (bt, ot); allocate ot-major so the
        # deferred-ot tiles are the last slots claimed from the pool
        ps = [[None] * n_ot for _ in range(BT)]
        for ot in range(n_ot):
            for bt in range(BT):
                ps[bt][ot] = ppool.tile([128, 512], F32, tag="psum",
                                        name=f"ps_l{li}_b{bt}_o{ot}")

        def seed(ot):
            # blended bias seeds the accumulation group (L2/L3 only)
            otw = min(128, dout - ot * 128)
            for bt in range(BT):
                nc.tensor.matmul(
                    ps[bt][ot][0:otw, :],
                    bias[:, boff + ot * 128: boff + ot * 128 + otw],
                    blend[:, bass.ts(bt, BW)],
                    start=True, stop=False,
                )

        def mm(e, kc, ot, rhs, last):
            otw = min(128, dout - ot * 128)
            for bt in range(BT):
                nc.tensor.matmul(
                    ps[bt][ot][0:otw, :], wsl(e, kc, ot, otw),
                    rhs[:, bass.ts(bt, BW)],
                    start=(boff is None and e == 0 and kc == 0),
                    stop=last,
                )

        if boff is not None:
            for ot in range(ol):
                seed(ot)

        # expert 0: run all K-chunks over the non-deferred output tiles
        # first, then touch the deferred tile; this gives the previous
        # layer's last PSUM users (softmax sums at the rep boundary) time
        # to drain without stalling the PE
        rhs0 = prebuilt or [build_rhs(0, kc, f"rhs_l{li}_e0_k{kc}")
                            for kc in range(KC)]
        if not tail_done:
            # previous iteration's softmax normalization + writeout; the
            # DVE/Pool/ACT work here hides under the L1 expert loop
            _softmax_tail(nc, dpool, exs_p, ssb_p, y_d)
            tail_done = True
        for kc in range(KC):
            for ot in range(ol):
                mm(0, kc, ot, rhs0[kc], last=False)
        if boff is not None:
            seed(ol)
        for kc in range(KC):
            mm(0, kc, ol, rhs0[kc], last=False)

        for e in range(1, E - 1):
            for kc in range(KC):
                rhs = build_rhs(e, kc, f"rhs_l{li}_e{e}_k{kc}")
                for ot in range(n_ot):
                    mm(e, kc, ot, rhs, last=False)

        # last expert: ot-outer so each accumulator finishes a full K-round
        # early; its drain pipelines behind the remaining matmul blocks and
        # the next layer's e0 rhs tiles are built inside the drain sequence
        e = E - 1
        rhs7 = [build_rhs(e, kc, f"rhs_l{li}_e{e}_k{kc}") for kc in range(KC)]
        hnext = srcs[li + 1] if li < 2 else None
        nxt = []
        sms = []
        if li == 2:
            for bt in range(BT):
                sms.append(ppool.tile([128, 512], F32, tag="psum",
                                      name=f"sm_b{bt}"))
        for ot in range(n_ot):
            otw = min(128, dout - ot * 128)
            for kc in range(KC):
                mm(e, kc, ot, rhs7[kc], last=(kc == KC - 1))
            if li < 2:
                # ELU drain of this output tile into the next layer's
                # transposed fp16 activations (tile ot is K-chunk ot there)
                for bt in range(BT):
                    bsl = bass.ts(bt, BW)
                    p = ps[bt][ot]
                    et = dpool.tile([128, BW], F32, tag="et", bufs=4,
                                    name=f"et_l{li}_b{bt}_o{ot}")
                    nc.scalar.activation(et[:], p[:], AF.Exp)
                    # et = min(exp(v) - 1, 0) on GPSIMD (keeps DVE short)
                    nc.vector.tensor_scalar(
                        et[:], et[:], 1.0, 0.0, OP.subtract, OP.min)
                    # h = max(v, 0) + et  (fp16 out)
                    nc.vector.scalar_tensor_tensor(
                        hnext[:, ot, bsl], p[:], 0.0, et[:],
                        OP.max, OP.add)
                nxt.append(build_rhs_next(nc, rpool, srcs[li + 1], bcast,
                                          ot, li))
            else:
                # softmax partial: exp then accumulate the partition sum
                # via a ones-stationary f32r matmul
                for bt in range(BT):
                    ex = exs_p[bt][ot]
                    nc.scalar.activation(
                        ex[0:otw, :].bitcast(F32R),
                        ps[bt][ot][0:otw, :], AF.Exp)
                    nc.tensor.matmul(
                        sms[bt][0:1, :], ones[0:otw, 0:1],
                        ex[0:otw, :].bitcast(F32R),
                        start=(ot == 0), stop=(ot == n_ot - 1),
                    )
        prebuilt = nxt if li < 2 else None

    # bank the softmax sums in SBUF (ACT copy frees the PSUM banks); the
    # normalization itself is deferred to the next iteration / epilogue
    for bt in range(BT):
        nc.scalar.activation(ssb_p[bt][:], sms[bt][0:1, :], AF.Identity)



def _softmax_tail(nc, dpool, exs_p, ssb_p, y_d):
    """Normalize and write out the previous iteration's softmax state."""
    dout, n_ot, _ = LAYERS[2]
    denbs = []
    for bt in range(BT):
        recip = dpool.tile([1, BW], F32, tag="recip", bufs=2,
                           name=f"recip_b{bt}")
        nc.vector.reciprocal(recip[:], ssb_p[bt][:])
        denb = dpool.tile([128, BW], F32, tag="denb", bufs=2,
                          name=f"denb_b{bt}")
        nc.gpsimd.partition_broadcast(denb[:], recip[:])
        denbs.append(denb)
    for ot in range(n_ot):
        otw = min(128, dout - ot * 128)
        for bt in range(BT):
            yt = dpool.tile([128, BW], F32, tag="yt", bufs=3,
                            name=f"yt_b{bt}_o{ot}")
            nc.vector.tensor_tensor(
                yt[0:otw, :], exs_p[bt][ot][0:otw, :], denbs[bt][0:otw, :],
                OP.mult)
            nc.sync.dma_start(
                out=y_d[ot * 128: ot * 128 + otw, bass.ts(bt, BW)],
                in_=yt[0:otw, :])


def build_rhs_next(nc, rpool, hnext, bcast, kc, li):
    """e0 rhs tile for the next layer, built as soon as h[:, kc, :] lands."""
    rhs = rpool.tile([128, BC], F16, tag="rhs", name=f"rhs_l{li + 1}_e0_k{kc}")
    nc.vector.tensor_tensor(rhs[:], hnext[:, kc, :], bcast[:, 0, :], OP.mult)
    return rhs


_NC_CACHE = {}


def _get_program(reps=1):
    if reps not in _NC_CACHE:
        _NC_CACHE[reps] = _build_program(reps)
    return _NC_CACHE[reps]


def _prep_inputs(x, weight_blend, W1, b1, W2, b2, W3, b3):
    x = np.asarray(x, np.float32)
    blend = np.asarray(weight_blend, np.float32)

    xp = np.zeros((B, D0P), np.float32)
    xp[:, :D0] = x
    xp[:, D0] = 1.0                                      # bias row for L1
    xT = np.ascontiguousarray(xp.T)                      # [512, B]

    def pack_w(W, din, brow=None):
        # W: (E, dout, din) -> [128, E*KC*dout], chunk (e,kc) at col (e*KC+kc)*dout
        Wt = np.zeros((E, KC * 128, W.shape[1]), np.float32)
        Wt[:, :din, :] = np.transpose(W, (0, 2, 1))
        if brow is not None:
            Wt[:, din, :] = brow
        # (E, KC, 128, dout) -> (128, E, KC, dout)
        return np.ascontiguousarray(
            Wt.reshape(E, KC, 128, W.shape[1])
            .transpose(2, 0, 1, 3)
            .reshape(128, -1)).astype(np.float16)

    w1h = pack_w(np.asarray(W1, np.float32), D0, np.asarray(b1, np.float32))
    w2h = pack_w(np.asarray(W2, np.float32), D1)
    w3h = pack_w(np.asarray(W3, np.float32), D2)
    bias_h = np.concatenate(
        [np.asarray(b2, np.float32), np.asarray(b3, np.float32)],
        axis=1).astype(np.float16)
    ones_h = np.ones((128, 1), np.float32)

    in_maps = []
    for c in range(N_CORES):
        csl = slice(c * BC, (c + 1) * BC)
        xt_c = np.ascontiguousarray(
            xT[:, csl].reshape(KC, 128, BC).transpose(1, 0, 2)
            .reshape(128, -1)).astype(np.float16)
        bl_c = np.ascontiguousarray(blend[:, csl])
        bc_c = np.ascontiguousarray(
            np.broadcast_to(bl_c[None, :, :], (128, E, BC))
            .reshape(128, -1)).astype(np.float16)
        in_maps.append({
            "xt": xt_c,
            "bcast": bc_c,
            "blend": bl_c.astype(np.float16),
            "bias": bias_h,
            "ones": ones_h,
            "w1": w1h, "w2": w2h, "w3": w3h,
        })
    return in_maps


def run(inputs, trace=False, trace_kwargs=None, reps=1):
    nc = _get_program(reps)
    in_maps = _prep_inputs(
        inputs["x"], inputs["weight_blend"],
        inputs["W1"], inputs["b1"], inputs["W2"], inputs["b2"],
        inputs["W3"], inputs["b3"])
    res = run_bass_kernel_spmd(
        nc, in_maps, list(range(N_CORES)),
        trace=trace, **(trace_kwargs or {}))
    y = np.concatenate([res.results[c]["y"] for c in range(N_CORES)], axis=1)
    return np.ascontiguousarray(y.T), res


def kernel(**inputs):
    y, _ = run(inputs, trace=False)
    return y


# revision 16
# speedup vs baseline: 14.5453x; 14.5453x over previous
"""Trainium2 Bass kernel for blended-expert 3-layer MLP (moe_routing).

Math (per sample b):
  h1 = elu(sum_e blend[e,b] * (W1[e] @ x[b]  + b1[e]))
  h2 = elu(sum_e blend[e,b] * (W2[e] @ h1[b] + b2[e]))
  y  = softmax(sum_e blend[e,b] * (W3[e] @ h2[b] + b3[e]))

Strategy (per core, data-parallel over batch: B=8192 -> Bc=1024 per core):
  - fp16 everywhere on the matmul path (10-bit mantissa ~ fp32r's 11):
    weights live fp16 and fully SBUF-RESIDENT (13.3MB), loaded once in the
    prologue, so the steady-state loop does no weight DMA at all.
  - Activations live TRANSPOSED in SBUF as fp16: hT[d, b] (d on partitions).
    Host pre-transposes x; host un-transposes the [363, Bc] output.
  - Blended linear as one PSUM accumulation: for each expert e the moving
    operand is rhs_e = hT * blend[e, :] (DVE tensor_tensor in fp16 2x mode
    with a host-replicated broadcast tile), the stationary is a chunk of
    W_e^T (fp16, 1 cycle/row on PE - same as fp32r).
  - L1 bias rides the padded K dim: x is padded 480->512 with row 480 = 1,
    and the W1 pack puts b1[e] in row 480, so rhs row 480 = blend[e,:] and
    the bias accumulates inside the main matmuls for free.
  - L2/L3 blended bias enters the PSUM group via a K=8 fp16 matmul
    (stationary = bias matrix [8, out], moving = blend [8, b]).
  - ELU drain: ACT exp + DVE tensor_scalar/scalar_tensor_tensor:
      elu(v) = max(v, 0) + min(exp(v) - 1, 0)     -> written as fp16
  - Softmax stays fp32 (exp can exceed fp16 range): exp via ACT, partition
    sums via a ones-stationary f32r matmul. The normalization (reciprocal,
    GPSIMD partition-broadcast, scale, DMA out) is SKEWED one iteration:
    iteration i banks its exp tiles and sums in SBUF, iteration i+1 (or an
    epilogue flush) normalizes them while its own L1 matmuls run, so the
    tail never blocks the PE or the DVE rhs pipeline.
  - Software pipelining: each layer's last expert runs ot-outer so every
    accumulator finishes a full K-round early; its ELU drain and the next
    layer's e0 rhs build pipeline behind the remaining matmul blocks. With
    this the sim shows the PE 100% busy in steady state (~156us/rep vs the
    ~170us/rep of the fp32r streaming baseline).
  - PSUM: 8 banks exactly. The last-ot accumulator of each layer is seeded
    and first-touched only after a full e0 K-round of the other accumulators,
    so the two banks that the previous softmax sums occupy have time to free
    without stalling the PE at layer/rep boundaries.
"""

import numpy as np

import concourse.bass as bass
import concourse.mybir as mybir
import concourse.tile as tile
from concourse import bacc
from concourse.bass_utils import run_bass_kernel_spmd

F32 = mybir.dt.float32
F32R = mybir.dt.float32r
F16 = mybir.dt.float16
AF = mybir.ActivationFunctionType
OP = mybir.AluOpType

N_CORES = 8
E = 8
B = 8192
BC = B // N_CORES          # 1024 per core
BT = 2                     # batch halves per core (PSUM free dim = 512)
BW = BC // BT              # 512
D0, D1, D2, D3 = 480, 512, 512, 363
D0P = 512                  # input dim padded to 4 K-chunks of 128
KC = 4                     # K chunks of 128 per expert (all layers)
# layer table: (out_dim, n_otiles, bias column offset into bias tile or None)
LAYERS = [(D1, 4, None), (D2, 4, 0), (D3, 3, D2)]
BIAS_W = D2 + D3           # b1 is folded into the W1 pack


def _build_program(reps=1, unroll=False):
    nc = bacc.Bacc("TRN2", target_bir_lowering=False, debug=False,
                   num_devices=N_CORES)

    xt_d = nc.dram_tensor("xt", [128, KC * BC], F16, kind="ExternalInput").ap()
    bc_d = nc.dram_tensor("bcast", [128, E * BC], F16, kind="ExternalInput").ap()
    bl_d = nc.dram_tensor("blend", [E, BC], F16, kind="ExternalInput").ap()
    bias_d = nc.dram_tensor("bias", [E, BIAS_W], F16, kind="ExternalInput").ap()
    ones_d = nc.dram_tensor("ones", [128, 1], F32R, kind="ExternalInput").ap()
    w_d = [
        nc.dram_tensor("w1", [128, E * KC * D1], F16, kind="ExternalInput").ap(),
        nc.dram_tensor("w2", [128, E * KC * D2], F16, kind="ExternalInput").ap(),
        nc.dram_tensor("w3", [128, E * KC * D3], F16, kind="ExternalInput").ap(),
    ]
    y_d = nc.dram_tensor("y", [D3, BC], F32, kind="ExternalOutput").ap()

    with tile.TileContext(nc) as tc:
        with (
            tc.tile_pool(name="const", bufs=1) as cpool,
            tc.tile_pool(name="acts", bufs=1) as apool,
            tc.tile_pool(name="rhs", bufs=10) as rpool,
            tc.tile_pool(name="drain", bufs=4) as dpool,
            tc.tile_pool(name="psum", bufs=8, space="PSUM") as ppool,
        ):
            xt = cpool.tile([128, KC, BC], F16)
            bcast = cpool.tile([128, E, BC], F16)
            blend = cpool.tile([E, BC], F16)
            bias = cpool.tile([E, BIAS_W], F16)
            ones = cpool.tile([128, 1], F32R)
            ws = [
                cpool.tile([128, E * KC * D1], F16, name="w1s"),
                cpool.tile([128, E * KC * D2], F16, name="w2s"),
                cpool.tile([128, E * KC * D3], F16, name="w3s"),
            ]
            # prologue: everything is resident; spread the ~13MB of loads
            # over both the SP and Activation HW-DGE queues
            nc.sync.dma_start(out=blend[:], in_=bl_d[:])
            nc.sync.dma_start(out=bias[:], in_=bias_d[:])
            nc.sync.dma_start(out=ones[:], in_=ones_d[:])
            nc.sync.dma_start(out=xt[:], in_=xt_d[:])
            nc.sync.dma_start(out=bcast[:], in_=bc_d[:])
            nc.sync.dma_start(out=ws[0][:], in_=w_d[0][:])
            nc.scalar.dma_start(out=ws[1][:], in_=w_d[1][:])
            nc.scalar.dma_start(out=ws[2][:], in_=w_d[2][:])

            h1 = apool.tile([128, KC, BC], F16)
            h2 = apool.tile([128, KC, BC], F16)
            srcs = [xt, h1, h2]
            # persistent softmax state: iteration i's exp tiles / sums are
            # normalized and written out at the START of iteration i+1 (the
            # DVE is idle there), with one epilogue flush after the loop
            exs_p = [[apool.tile([128, BW], F32, name=f"exp_b{bt}_o{ot}")
                      for ot in range(LAYERS[2][1])] for bt in range(BT)]
            ssb_p = [apool.tile([1, BW], F32, name=f"ssb_b{bt}")
                     for bt in range(BT)]
            for bt in range(BT):
                nc.gpsimd.memset(ssb_p[bt][:], 1.0)
                for ot in range(LAYERS[2][1]):
                    nc.gpsimd.memset(exs_p[bt][ot][:], 0.0)

            def body():
                _network(nc, tc, srcs, bcast, blend, bias, ones,
                         ws, y_d, rpool, dpool, ppool, exs_p, ssb_p)

            if reps == 1:
                body()
            elif unroll:
                for _ in range(reps):
                    body()
            else:
                with tc.For_i(0, reps, 1):
                    body()
            _softmax_tail(nc, dpool, exs_p, ssb_p, y_d)
    nc.compile()
    return nc


def _network(nc, tc, srcs, bcast, blend, bias, ones, ws, y_d,
             rpool, dpool, ppool, exs_p, ssb_p):
    tail_done = False
    prebuilt = None          # e0 rhs tiles built during the previous layer
    for li, (dout, n_ot, boff) in enumerate(LAYERS):
        src = srcs[li]
        w = ws[li]
        ol = n_ot - 1          # deferred (last) output tile

        def wsl(e, kc, ot, otw):
            base = (e * KC + kc) * dout + ot * 128
            return w[:, base:base + otw]

        def build_rhs(e, kc, name):
            rhs = rpool.tile([128, BC], F16, tag="rhs", name=name)
            nc.vector.tensor_tensor(rhs[:], src[:, kc, :], bcast[:, e, :],
                                    OP.mult)
            return rhs

        # psum accumulators: one bank per (bt, ot); allocate ot-major so the
        # deferred-ot tiles are the last slots claimed from the pool
        ps = [[None] * n_ot for _ in range(BT)]
        for ot in range(n_ot):
            for bt in range(BT):
                ps[bt][ot] = ppool.tile([128, 512], F32, tag="psum",
                                        name=f"ps_l{li}_b{bt}_o{ot}")

        def seed(ot):
            # blended bias seeds the accumulation group (L2/L3 only)
            otw = min(128, dout - ot * 128)
            for bt in range(BT):
                nc.tensor.matmul(
                    ps[bt][ot][0:otw, :],
                    bias[:, boff + ot * 128: boff + ot * 128 + otw],
                    blend[:, bass.ts(bt, BW)],
                    start=True, stop=False,
                )

        def mm(e, kc, ot, get_rhs, last):
            otw = min(128, dout - ot * 128)
            for bt in range(BT):
                nc.tensor.matmul(
                    ps[bt][ot][0:otw, :], wsl(e, kc, ot, otw),
                    get_rhs(bt),
                    start=(boff is None and e == 0 and kc == 0),
                    stop=last,
                )

        def sl2(t):
            return lambda bt: t[:, bass.ts(bt, BW)]

        if boff is not None:
            for ot in range(ol):
                seed(ot)

        # expert 0: run all K-chunks over the non-deferred output tiles
        # first, then touch the deferred tile; this gives the previous
        # layer's last PSUM users (softmax sums at the rep boundary) time
        # to drain without stalling the PE
        if prebuilt is not None:
            rhs0 = [sl2(t) for t in prebuilt]
        else:
            rhs0 = [sl2(build_rhs(0, kc, f"rhs_l{li}_e0_k{kc}"))
                    for kc in range(KC)]
        if not tail_done:
            # previous iteration's softmax normalization + writeout; the
            # DVE/Pool/ACT work here hides under the L1 expert loop
            _softmax_tail(nc, dpool, exs_p, ssb_p, y_d)
            tail_done = True
        for kc in range(KC):
            for ot in range(ol):
                mm(0, kc, ot, rhs0[kc], last=False)
        if boff is not None:
            seed(ol)
        for kc in range(KC):
            mm(0, kc, ol, rhs0[kc], last=False)

        for e in range(1, E - 1):
            for kc in range(KC):
                r = sl2(build_rhs(e, kc, f"rhs_l{li}_e{e}_k{kc}"))
                for ot in range(n_ot):
                    mm(e, kc, ot, r, last=False)

        # last expert: ot-outer so each accumulator finishes a full K-round
        # early; its drain pipelines behind the remaining matmul blocks and
        # the next layer's e0 rhs tiles are built inside the drain sequence
        e = E - 1
        rhs7 = [sl2(build_rhs(e, kc, f"rhs_l{li}_e{e}_k{kc}"))
                for kc in range(KC)]
        hnext = srcs[li + 1] if li < 2 else None
        nxt = []
        sms = []
        if li == 2:
            for bt in range(BT):
                sms.append(ppool.tile([128, 512], F32, tag="psum",
                                      name=f"sm_b{bt}"))
        for ot in range(n_ot):
            otw = min(128, dout - ot * 128)
            for kc in range(KC):
                mm(e, kc, ot, rhs7[kc], last=(kc == KC - 1))
            if li < 2:
                # ELU drain of this output tile into the next layer's
                # transposed fp16 activations (tile ot is K-chunk ot there)
                for bt in range(BT):
                    bsl = bass.ts(bt, BW)
                    p = ps[bt][ot]
                    et = dpool.tile([128, BW], F32, tag="et", bufs=4,
                                    name=f"et_l{li}_b{bt}_o{ot}")
                    nc.scalar.activation(et[:], p[:], AF.Exp)
                    # et = min(exp(v) - 1, 0) on GPSIMD (keeps DVE short)
                    nc.vector.tensor_scalar(
                        et[:], et[:], 1.0, 0.0, OP.subtract, OP.min)
                    # h = max(v, 0) + et  (fp16 out)
                    nc.vector.scalar_tensor_tensor(
                        hnext[:, ot, bsl], p[:], 0.0, et[:],
                        OP.max, OP.add)
                nxt.append(build_rhs_next(nc, rpool, srcs[li + 1], bcast,
                                          ot, li))
            else:
                # softmax partial: exp then accumulate the partition sum
                # via a ones-stationary f32r matmul
                for bt in range(BT):
                    ex = exs_p[bt][ot]
                    nc.scalar.activation(
                        ex[0:otw, :].bitcast(F32R),
                        ps[bt][ot][0:otw, :], AF.Exp)
                    nc.tensor.matmul(
                        sms[bt][0:1, :], ones[0:otw, 0:1],
                        ex[0:otw, :].bitcast(F32R),
                        start=(ot == 0), stop=(ot == n_ot - 1),
                    )
        prebuilt = nxt if li < 2 else None

    # bank the softmax sums in SBUF (ACT copy frees the PSUM banks); the
    # normalization itself is deferred to the next iteration / epilogue
    for bt in range(BT):
        nc.scalar.activation(ssb_p[bt][:], sms[bt][0:1, :], AF.Identity)



def _softmax_tail(nc, dpool, exs_p, ssb_p, y_d):
    """Normalize and write out the previous iteration's softmax state."""
    dout, n_ot, _ = LAYERS[2]
    denbs = []
    for bt in range(BT):
        recip = dpool.tile([1, BW], F32, tag="recip", bufs=2,
                           name=f"recip_b{bt}")
        nc.vector.reciprocal(recip[:], ssb_p[bt][:])
        denb = dpool.tile([128, BW], F32, tag="denb", bufs=2,
                          name=f"denb_b{bt}")
        nc.gpsimd.partition_broadcast(denb[:], recip[:])
        denbs.append(denb)
    for ot in range(n_ot):
        otw = min(128, dout - ot * 128)
        for bt in range(BT):
            yt = dpool.tile([128, BW], F32, tag="yt", bufs=3,
                            name=f"yt_b{bt}_o{ot}")
            nc.vector.tensor_tensor(
                yt[0:otw, :], exs_p[bt][ot][0:otw, :], denbs[bt][0:otw, :],
                OP.mult)
            nc.sync.dma_start(
                out=y_d[ot * 128: ot * 128 + otw, bass.ts(bt, BW)],
                in_=yt[0:otw, :])


def build_rhs_next(nc, rpool, hnext, bcast, kc, li):
    """e0 rhs tile for the next layer, built as soon as h[:, kc, :] lands."""
    rhs = rpool.tile([128, BC], F16, tag="rhs", name=f"rhs_l{li + 1}_e0_k{kc}")
    nc.vector.tensor_tensor(rhs[:], hnext[:, kc, :], bcast[:, 0, :], OP.mult)
    return rhs


_NC_CACHE = {}


def _get_program(reps=1):
    if reps not in _NC_CACHE:
        _NC_CACHE[reps] = _build_program(reps)
    return _NC_CACHE[reps]


def _prep_inputs(x, weight_blend, W1, b1, W2, b2, W3, b3):
    x = np.asarray(x, np.float32)
    blend = np.asarray(weight_blend, np.float32)

    xp = np.zeros((B, D0P), np.float32)
    xp[:, :D0] = x
    xp[:, D0] = 1.0                                      # bias row for L1
    xT = np.ascontiguousarray(xp.T)                      # [512, B]

    def pack_w(W, din, brow=None):
        # W: (E, dout, din) -> [128, E*KC*dout], chunk (e,kc) at col (e*KC+kc)*dout
        Wt = np.zeros((E, KC * 128, W.shape[1]), np.float32)
        Wt[:, :din, :] = np.transpose(W, (0, 2, 1))
        if brow is not None:
            Wt[:, din, :] = brow
        # (E, KC, 128, dout) -> (128, E, KC, dout)
        return np.ascontiguousarray(
            Wt.reshape(E, KC, 128, W.shape[1])
            .transpose(2, 0, 1, 3)
            .reshape(128, -1)).astype(np.float16)

    w1h = pack_w(np.asarray(W1, np.float32), D0, np.asarray(b1, np.float32))
    w2h = pack_w(np.asarray(W2, np.float32), D1)
    w3h = pack_w(np.asarray(W3, np.float32), D2)
    bias_h = np.concatenate(
        [np.asarray(b2, np.float32), np.asarray(b3, np.float32)],
        axis=1).astype(np.float16)
    ones_h = np.ones((128, 1), np.float32)

    in_maps = []
    for c in range(N_CORES):
        csl = slice(c * BC, (c + 1) * BC)
        xt_c = np.ascontiguousarray(
            xT[:, csl].reshape(KC, 128, BC).transpose(1, 0, 2)
            .reshape(128, -1)).astype(np.float16)
        bl_c = np.ascontiguousarray(blend[:, csl])
        bc_c = np.ascontiguousarray(
            np.broadcast_to(bl_c[None, :, :], (128, E, BC))
            .reshape(128, -1)).astype(np.float16)
        in_maps.append({
            "xt": xt_c,
            "bcast": bc_c,
            "blend": bl_c.astype(np.float16),
            "bias": bias_h,
            "ones": ones_h,
            "w1": w1h, "w2": w2h, "w3": w3h,
        })
    return in_maps


def run(inputs, trace=False, trace_kwargs=None, reps=1):
    nc = _get_program(reps)
    in_maps = _prep_inputs(
        inputs["x"], inputs["weight_blend"],
        inputs["W1"], inputs["b1"], inputs["W2"], inputs["b2"],
        inputs["W3"], inputs["b3"])
    res = run_bass_kernel_spmd(
        nc, in_maps, list(range(N_CORES)),
        trace=trace, **(trace_kwargs or {}))
    y = np.concatenate([res.results[c]["y"] for c in range(N_CORES)], axis=1)
    return np.ascontiguousarray(y.T), res


def kernel(**inputs):
    y, _ = run(inputs, trace=False)
    return y
